# revision 1
# baseline (speedup 1.0000x reference)
"""Trainium2 Bass kernel: conv3d(16,3x3x3,VALID) -> channel softmax -> 2x maxpool3d(2) twice.

Full inputs: x [8,3,96,96,96] f32, w [16,3,3,3,3] f32, b [16] f32.
Output: [8,16,23,23,23] f32.

Sharding: data-parallel over batch N=8 across 8 NeuronCores (1 sample/core).

Per-core algorithm (sample x_i [3,96,96,96] -> out_i [16,23,23,23]):
  Only conv outputs d,h,w in [0,92) survive the two maxpools (23*4=92), so we
  compute conv on a 92^3 grid, grouped as 23 d-quads x 23 h-quads x 92 w.

  Layout trick: one PSUM tile [128, 368] holds 8 h-quads x 16 channels on the
  partition axis (partition p = 16*g + c, g = h-quad index within a chunk of 8)
  and (h_local 4, w 92) on the free axis, for one conv-output depth d.
  The conv is a matmul with a block-diagonal stationary operand:
    lhsT [108, 64] = diag blocks of w[(cin,kd,kh), cout] per kw tap (K=27*4),
  accumulating 3 kw taps into PSUM via column-shifted views of one rhs tile.
  Two concurrent matmuls (tile_position col groups (0,0) and (0,64)) fill all
  128 partitions.

  Softmax+pool in log domain: y = (x+b) - ln(sum_c exp(x_c+b)) and
  maxpool(softmax) = exp(maxpool(y)) since exp is monotone. So:
    exp:  ACT e = exp(logits + b) -> SBUF as float32r (12-bit mantissa)
    sum:  PE  S = lambda * sum_c e, group-BROADCAST to all 128 partitions
          via a [128,128] block-ones lhsT; lambda = 2^-38 keeps ln's input
          inside the ACT Ln LUT's valid range [e^-44.5, e^44.5]
    ln:   ACT full-f32 ln(S) (f32r would round |ln S|~40 too coarsely)
    sub:  DVE y = logits - ln(S)
    pool: DVE reduce_max over w, then h_local, then d (all free-axis APs)
    out:  ACT exp(y_pool + b - 38*ln2), one 3-dim DMA per (dq, chunk).

  Conv matmuls are fp16 hi/lo 3-term (xh*wh + xh*wl + xl*wh, ~2e-5 rel):
  fp32 matmul is 4 cycles/row and float32r matmuls may only write PSUM
  partition 0 (ISA s3d3 check), which would kill the col-group concurrency.
  The sum matmul stays float32r at base partition 0 (1 cycle/row, N>=256).

  DMA: each dma_start costs ~0.76us fixed regardless of size, and DMA APs
  are limited to 3 dims with a contiguous last dim. So x is first staged
  into B[27 taps, d, h, w] in DRAM (54 big HBM->HBM copies, both HWDGE
  rings), after which each rhs im2col tile quarter loads as a single
  27-partition DMA (288 loads total instead of 2592 3-partition pieces).
"""

import numpy as np
from contextlib import ExitStack

import concourse.bass as bass
import concourse.bacc as bacc
import concourse.tile as tile
from concourse import mybir
from concourse.bass_utils import run_bass_kernel_spmd

F32 = mybir.dt.float32
F32R = mybir.dt.float32r
BF16 = mybir.dt.bfloat16
F16 = mybir.dt.float16

N_CORES = 8
CIN, S = 3, 96
COUT = 16
Q = 23          # pooled output size per dim
DU = 92         # conv positions used per dim (23*4)
NW = 94         # w extent loaded (92 + 2 halo for kw shifts)

CONV_MODE = "hilo"   # "f32r" | "hilo" (fp16 hi/lo 3-term)

_cache: dict = {}


def _emit(nc, xs, wls, ws_, wbc_, bias_, bias2_, out_, mode, chunks=(0, 8, 15),
          dq0s=None, ndq_cap=4, repeat=1, stage="full"):
    """Build the Tile kernel. xs: list of x dram APs (1 for f32r, 2 for bf16x2
    [hi, lo]); wls: list of lhsT dram APs ([108,3,64] each)."""
    S2 = S * S          # 9216
    S3 = S * S * S      # 884736
    if dq0s is None:
        dq0s = range(0, Q, 4)

    with tile.TileContext(nc) as tc, ExitStack() as ctx:
        consts = ctx.enter_context(tc.tile_pool(name="consts", bufs=1))
        rhsp = ctx.enter_context(tc.tile_pool(name="rhs", bufs=3))
        ep = ctx.enter_context(tc.tile_pool(name="e", bufs=3))
        ellp = ctx.enter_context(tc.tile_pool(name="ell", bufs=3))
        yp = ctx.enter_context(tc.tile_pool(name="y", bufs=3))
        wpp = ctx.enter_context(tc.tile_pool(name="wp", bufs=2))
        hpp = ctx.enter_context(tc.tile_pool(name="hp", bufs=2))
        finp = ctx.enter_context(tc.tile_pool(name="fin", bufs=2))
        outp = ctx.enter_context(tc.tile_pool(name="outt", bufs=2))
        psl = ctx.enter_context(tc.tile_pool(name="psl", bufs=3, space="PSUM"))
        pss = ctx.enter_context(tc.tile_pool(name="pss", bufs=1, space="PSUM"))

        rhs_dt = F32R if mode == "f32r" else F16
        
        # constants
        wlts = []
        for i, wl in enumerate(wls):
            t = consts.tile([108, 3, 64], rhs_dt, tag=f"wl{i}")
            nc.sync.dma_start(out=t, in_=wl[:])
            wlts.append(t)
        wst = consts.tile([128, 128], F32R, tag="ws")
        nc.sync.dma_start(out=wst, in_=ws_[:])
        biast = consts.tile([128, 1], F32, tag="bias")
        nc.sync.dma_start(out=biast, in_=bias_[:])
        biast2 = consts.tile([128, 1], F32, tag="bias2")
        nc.sync.dma_start(out=biast2, in_=bias2_[:])

        nx = len(xs)  # matmul terms per tap (1 or 3 -> hi/lo operand pairs)
        # term -> (x operand index, lhsT operand index)
        if mode == "f32r":
            terms = [(0, 0)]
        else:
            terms = [(0, 0), (0, 1), (1, 0)]  # xh*wh + xh*wl + xl*wh

        # --- staging: B[v][slot=(ci,kd,kh), d, h, w] = x[ci, d+kd, h+kh, w] ---
        # One HBM->HBM copy per slot (27 per operand). After staging, each
        # rhs tile quarter loads with a single 27-partition DMA (the 3-dim
        # DMA AP limit makes direct strided loads need 3-partition pieces,
        # and each dma_start costs ~0.76us fixed).
        DH, HH, HW = 94, 93, 96
        dramp = ctx.enter_context(tc.tile_pool(name="dram", bufs=1, space="DRAM"))
        Bs = []
        for v in range(nx):
            bt = dramp.tile([27, DH, HH, HW], rhs_dt, tag=f"B{v}")
            for ci in range(CIN):
                for kd in range(3):
                    for kh in range(3):
                        slot = 9 * ci + 3 * kd + kh
                        src = bass.AP(
                            tensor=xs[v],
                            offset=ci * S3 + kd * S2 + kh * S,
                            ap=[[S2, DH], [1, HH * HW]],
                        )
                        eng = nc.scalar if (slot % 2) else nc.sync
                        eng.dma_start(
                            out=bt[slot].rearrange("d h w -> d (h w)"), in_=src)
            Bs.append(bt)

        for _rep in range(repeat):
          for hq0 in chunks:
            for dq0 in dq0s:
                ndq = min(ndq_cap, Q - dq0)
                E = 4 * ndq  # depths staged in this rhs tile group (16 or 12)
                # --- load rhs im2col tiles [108, E, 4*96] for 4 d-quads ---
                # rhs[v][a]: operand v (hi/lo), half a (h-quad groups 4a..4a+3)
                # partition r = 27*g4 + 9*ci + 3*kd + kh; free = (d, h_local*96+w)
                EL = 1 if stage == "dmat" else E
                rhs = [[None, None] for _ in range(nx)]
                for v in range(nx):
                    for a in (0, 1):
                        t = rhsp.tile([108, 16, 4 * S], rhs_dt, tag=f"rhs{v}{a}")
                        rhs[v][a] = t
                        for g4 in range(4):
                            hq = hq0 + 4 * a + g4
                            src = bass.AP(
                                tensor=Bs[v].tensor,
                                offset=(Bs[v].offset
                                        + (4 * dq0) * HH * HW + (4 * hq) * HW),
                                ap=[[DH * HH * HW, 27], [HH * HW, EL], [1, 4 * S]],
                            )
                            eng = nc.scalar if (g4 % 2) else nc.sync
                            eng.dma_start(
                                out=t[27 * g4:27 * g4 + 27, 0:EL, :], in_=src)

                for dq in range(dq0, dq0 + ndq):
                    dsi0 = 4 * (dq - dq0)
                    if stage in ("dmao", "dmat"):
                        continue
                    hp = hpp.tile([128, 4, Q], F32)
                    if stage == "dma":
                        nc.vector.memset(hp, 0.0)
                    for pr in ((0, 1) if stage not in ("dma", "dmao") else ()):
                        logits = psl.tile([128, 2, 512], F32)
                        first = {(a, dl): True for a in (0, 1) for dl in (0, 1)}
                        nmm = 3 * len(terms)
                        cnt = {(a, dl): 0 for a in (0, 1) for dl in (0, 1)}
                        for dl in (0, 1):
                            dsi = dsi0 + 2 * pr + dl
                            for kw in range(3):
                                for a in (0, 1):
                                    for (xi, wi) in terms:
                                        lhsT = wlts[wi][:, kw, :]
                                        r = rhs[xi][a][:, dsi, :].rearrange(
                                            "p (hl w) -> p hl w", hl=4,
                                        )[:, :, kw:kw + DU]
                                        cnt[(a, dl)] += 1
                                        nc.tensor.matmul(
                                            out=logits[64 * a:64 * a + 64, dl, 0:368],
                                            lhsT=lhsT,
                                            rhs=r,
                                            start=first[(a, dl)],
                                            stop=(cnt[(a, dl)] == nmm),
                                            skip_group_check=True,
                                        )
                                        first[(a, dl)] = False
                        if stage == "conv":
                            wp0 = wpp.tile([128, 2, 4, Q], F32)
                            nc.vector.reduce_max(
                                out=wp0,
                                in_=logits[:, :, 0:368].rearrange(
                                    "p d (hl wq wl) -> p d hl wq wl",
                                    hl=4, wq=Q),
                                axis=mybir.AxisListType.X,
                            )
                            nc.vector.reduce_max(
                                out=hp[:, 2 * pr:2 * pr + 2, :],
                                in_=wp0.rearrange("p d hl wq -> p d wq hl"),
                                axis=mybir.AxisListType.X,
                            )
                            continue
                        # exp(logits + b) for both d of the pair, PSUM -> SBUF
                        e = ep.tile([128, 2, 368], F32R)
                        nc.scalar.activation(
                            out=e, in_=logits[:, :, 0:368],
                            func=mybir.ActivationFunctionType.Exp,
                            bias=biast[:, 0:1],
                        )
                        # per-group channel sums, broadcast to all 128
                        # partitions in one matmul: lhsT[k, p] = (k//16==p//16)
                        s = pss.tile([128, 2, 512], F32)
                        for dl in (0, 1):
                            nc.tensor.matmul(
                                out=s[:, dl, 0:368],
                                lhsT=wst,
                                rhs=e[:, dl, :],
                                start=True, stop=True,
                            )
                        # ln(s) in full fp32 (f32r would round |ln s|~40 too
                        # coarsely), then y = logits - ln(s) on DVE
                        ell = ellp.tile([128, 2, 368], F32)
                        nc.scalar.activation(
                            out=ell, in_=s[:, :, 0:368],
                            func=mybir.ActivationFunctionType.Ln,
                        )
                        y = yp.tile([128, 2, 368], F32)
                        nc.vector.tensor_tensor(
                            out=y, in0=logits[:, :, 0:368], in1=ell,
                            op=mybir.AluOpType.subtract,
                        )
                        # w-pool: [128, 2, 4, 23, 4] -> [128, 2, 4, 23]
                        wp = wpp.tile([128, 2, 4, Q], F32)
                        nc.vector.reduce_max(
                            out=wp,
                            in_=y.rearrange(
                                "p d (hl wq wl) -> p d hl wq wl", hl=4, wq=Q),
                            axis=mybir.AxisListType.X,
                        )
                        # h-pool: reduce over h_local -> hp[:, 2*pr:2*pr+2, :]
                        nc.vector.reduce_max(
                            out=hp[:, 2 * pr:2 * pr + 2, :],
                            in_=wp.rearrange("p d hl wq -> p d wq hl"),
                            axis=mybir.AxisListType.X,
                        )
                    # d-pool over the quad
                    fin = finp.tile([128, Q], F32)
                    nc.vector.reduce_max(
                        out=fin,
                        in_=hp.rearrange("p d wq -> p wq d"),
                        axis=mybir.AxisListType.X,
                    )
                    # back to probability domain, + bias inside exp
                    ot = outp.tile([128, Q], F32)
                    nc.scalar.activation(
                        out=ot, in_=fin,
                        func=mybir.ActivationFunctionType.Exp,
                        bias=biast2[:, 0:1],
                    )
                    if stage == "dmao":
                        continue
                    # SBUF side stays a plain [128, Q] AP (partition-major
                    # order is already g-major); the DRAM side carries the
                    # (g, c, w) pattern. Split-partition SBUF APs mislower.
                    dma_out_eng = nc.sync
                    dma_out_eng.dma_start(
                        out=out_[:][:, dq, hq0:hq0 + 8, :].rearrange(
                            "c g w -> g c w"),
                        in_=ot,
                    )


def _build(mode, chunks=(0, 8, 15), dq0s=None, ndq_cap=4, repeat=1, stage="full"):
    nc = bacc.Bacc(name="conv_softmax_pool")
    if mode == "f32r":
        xs = [nc.declare_dram_parameter("x", [CIN, S, S, S], F32R, isOutput=False)]
        wls = [nc.declare_dram_parameter("wl0", [108, 3, 64], F32R, isOutput=False)]
    else:
        xs = [
            nc.declare_dram_parameter("xh", [CIN, S, S, S], F16, isOutput=False),
            nc.declare_dram_parameter("xl", [CIN, S, S, S], F16, isOutput=False),
        ]
        wls = [
            nc.declare_dram_parameter("wl0", [108, 3, 64], F16, isOutput=False),
            nc.declare_dram_parameter("wl1", [108, 3, 64], F16, isOutput=False),
        ]
    ws_ = nc.declare_dram_parameter("ws", [128, 128], F32R, isOutput=False)
    wbc_ = None
    bias_ = nc.declare_dram_parameter("bias", [128, 1], F32, isOutput=False)
    bias2_ = nc.declare_dram_parameter("bias2", [128, 1], F32, isOutput=False)
    out_ = nc.declare_dram_parameter("out", [COUT, Q, Q, Q], F32, isOutput=True)
    _emit(nc, xs, wls, ws_, wbc_, bias_, bias2_, out_, mode, chunks=chunks,
          dq0s=dq0s, ndq_cap=ndq_cap, repeat=repeat, stage=stage)
    nc.finalize()
    return nc


def _host_prep(w, b, mode):
    """Build lhsT block-diagonal matrices and softmax helper matrices."""
    # wl[r, kw, m]: r = 27g + 9ci + 3kd + kh, m = 16g + c  (g = 0..3)
    def blockdiag(wm):  # wm [cout, cin, kd, kh, kw] float
        wl = np.zeros((108, 3, 64), np.float32)
        for g in range(4):
            for ci in range(CIN):
                for kd in range(3):
                    for kh in range(3):
                        wl[27 * g + 9 * ci + 3 * kd + kh, :, 16 * g:16 * g + 16] = \
                            wm[:, ci, kd, kh, :].T
        return wl

    # 2^-38 scale keeps ln(lambda*s) inside the ACT Ln LUT's valid input
    # range [e^-44.5, e^44.5]; compensated in the final exp bias.
    ws_ = np.zeros((128, 128), np.float32)
    for g in range(8):
        ws_[16 * g:16 * g + 16, 16 * g:16 * g + 16] = 2.0 ** -38
    wbc_ = None
    bias_ = np.tile(b.astype(np.float32), 8).reshape(128, 1)
    # y = logit - ln(lambda*s) = logit - ln s + 38ln2, so the final
    # exp needs bias2 = b - 38ln2 to recover exp(logit + b - ln s).
    bias2_ = bias_ - np.float32(38.0 * np.log(2.0))

    if mode == "f32r":
        wls = [blockdiag(w.astype(np.float32))]
    else:
        wh = w.astype(np.float32).astype(np.float16)
        wlo = (w.astype(np.float32) - wh.astype(np.float32)).astype(np.float16)
        wls = [blockdiag(wh.astype(np.float32)).astype(np.float16),
               blockdiag(wlo.astype(np.float32)).astype(np.float16)]
    return wls, ws_, wbc_, bias_, bias2_


def kernel(x, w, b):
    mode = CONV_MODE
    key = ("nc", mode)
    if key not in _cache:
        _cache[key] = _build(mode)
    nc = _cache[key]

    x = np.asarray(x, np.float32)
    w = np.asarray(w, np.float32)
    b = np.asarray(b, np.float32)
    wls, ws_, wbc_, bias_, bias2_ = _host_prep(w, b, mode)

    in_maps = []
    for i in range(N_CORES):
        m = {"ws": ws_, "bias": bias_, "bias2": bias2_}
        if mode == "f32r":
            m["x"] = np.ascontiguousarray(x[i])
            m["wl0"] = wls[0]
        else:
            xh = x[i].astype(np.float16)
            xl = (x[i] - xh.astype(np.float32)).astype(np.float16)
            m["xh"] = np.ascontiguousarray(xh)
            m["xl"] = np.ascontiguousarray(xl)
            m["wl0"] = wls[0]
            m["wl1"] = wls[1]
        in_maps.append(m)

    res = run_bass_kernel_spmd(nc, in_maps, core_ids=list(range(N_CORES)))
    return np.stack([r["out"] for r in res.results]).astype(np.float32)



# revision 19
# speedup vs baseline: 2.1568x; 2.1568x over previous
"""Trainium2 Bass kernel: conv3d(16ch,3x3x3,VALID) -> channel softmax -> 2x maxpool3d(2).

Full inputs: x [8,3,96,96,96] f32, w [16,3,3,3,3] f32, b [16] f32.
Output: [8,16,23,23,23] f32.  Data-parallel: 1 sample per NeuronCore.

Per-core pipeline (sample x_i [3,96,96,96] -> out_i [16,23,23,23]):
  Only conv outputs d,h,w in [0,92) survive the two maxpools (23*4=92).
  Partition layout: p = 16*g + c for 8 h-quads g and 16 channels c.
  Free layout per conv depth d: (hl 4, w 92) -> N=368 per matmul.

  conv: single-term fp16 (x, w both rounded to fp16; measured end-to-end
  max err 4.2e-3 vs the 2e-2 gate). lhsT [108,64] holds 4 diagonal
  [27,16] blocks (K = 27 taps (ci,kd,kh) per quad); kw handled by 3
  PSUM-accumulated matmuls on column-shifted rhs views. Two col-group
  matmuls (tile cols 0-63 / 64-127) cover 8 quads. Chunks of h-quads:
  (0,8,16); the last chunk has 7 quads and relies on lhsT[0:81,:,0:64]
  having zero cols 48-63 so partitions 112-127 get zeros, not stale PSUM.

  softmax+pool in log domain: y = (z+b) - ln S, maxpool(softmax) =
  exp(maxpool(y)).
    exp:  ACT e = exp(z + b) -> SBUF bf16
    sum:  PE lhsT ws32 [128,32] bf16 (lambda=2^-38 group indicators,
          4x duplicated cols) -> S at PSUM rows 32j..32j+32 for d-slice j.
          One [128,512] PSUM tile collects all 4 slices of a d-quad.
    ln:   two ACT Ln over the packed [128,368] tile: ->f16 gives hi,
          ->f32 gives the exact value; DVE subtract writes lo = ln-hi
          into rows 32j+8..16 (valid because S was duplicated there).
    sub:  one K=16 f16 matmul per slice accumulates -(hi+lo) onto the
          logits PSUM (lhsT bc16: -1 group indicators, rows 32j..32j+16).
    pool: DVE reduce_max over (hl,wl) per slice -> hp[:,j,:], then
          reduce over j -> fin [128,23]
    out:  ACT exp(fin + b - 38*ln2), one 3-dim DMA per (dq, chunk).

  DMA: x is staged once (9 HBM->HBM copies) into B27[27,94,8928] f16 with
  B27[9ci+3kd+kh][d][j] = x[ci,kd+d, kh*96+j]; each rhs im2col tile
  quarter then loads as one 27-partition DMA (innermost 768B, full rate).
"""

import numpy as np
from contextlib import ExitStack

import concourse.bass as bass
import concourse.bacc as bacc
import concourse.tile as tile
from concourse import mybir
from concourse.bass_utils import run_bass_kernel_spmd

F32 = mybir.dt.float32
BF16 = mybir.dt.bfloat16
F16 = mybir.dt.float16

N_CORES = 8
CIN, S = 3, 96
COUT = 16
Q = 23          # pooled output size per dim
DU = 92         # conv positions used per dim (23*4)
S2 = S * S      # 9216
S3 = S * S * S
HW27 = 93 * 96  # 8928: flattened (h,w) span per staged slot
BD = 94         # staged depth extent

_cache: dict = {}


def _emit(nc, xh, wl_, ws_, bc_, bias_, bias2_, out_, stage='full', prec='hilo'):
    # Preload the one ACT table set containing BOTH Exp and Ln; the
    # insert_act_table_loads pass then sees every activation covered and
    # emits no per-switch reloads (saves ~1.3us x ~276 switches).
    from concourse.hw_specs import get_activation_tables
    _tabs = list(get_activation_tables(nc.m.arch).keys())
    nc.scalar.add_instruction(mybir.InstLoadActFuncSet(
        name="preload_act",
        act_func_set_id=_tabs.index("natural_log_exp_and_others"),
        ins=[], outs=[]))
    with tile.TileContext(nc) as tc, ExitStack() as ctx:
        consts = ctx.enter_context(tc.tile_pool(name="consts", bufs=1))
        rhsp = ctx.enter_context(tc.tile_pool(name="rhs", bufs=2))
        ep = ctx.enter_context(tc.tile_pool(name="e", bufs=6))
        hlp = ctx.enter_context(tc.tile_pool(name="hl", bufs=2))
        lnp = ctx.enter_context(tc.tile_pool(name="ln", bufs=2))
        hpp = ctx.enter_context(tc.tile_pool(name="hp", bufs=2))
        finp = ctx.enter_context(tc.tile_pool(name="fin", bufs=2))
        outp = ctx.enter_context(tc.tile_pool(name="outt", bufs=2))
        psl = ctx.enter_context(tc.tile_pool(name="psl", bufs=6, space="PSUM"))
        pss = ctx.enter_context(tc.tile_pool(name="pss", bufs=2, space="PSUM"))
        dramp = ctx.enter_context(tc.tile_pool(name="dram", bufs=1, space="DRAM"))

        # constants
        wlt = consts.tile([108, 3, 64], F16, tag="wl")
        nc.sync.dma_start(out=wlt, in_=wl_[:])
        wst = consts.tile([128, 32], BF16, tag="ws")
        nc.sync.dma_start(out=wst, in_=ws_[:])
        bct = consts.tile([128, 128], F16, tag="bc")
        nc.sync.dma_start(out=bct, in_=bc_[:])
        biast = consts.tile([128, 1], F32, tag="bias")
        nc.sync.dma_start(out=biast, in_=bias_[:])
        biast2 = consts.tile([128, 1], F32, tag="bias2")
        nc.sync.dma_start(out=biast2, in_=bias2_[:])

        # --- staging: B27_k[9ci+3kd+kh, d, j] = xh[ci, kd+16k+d, kh*96 + j]
        # split into 6 d-chunk tiles so rhs loads of group k only wait for
        # chunk k's staging; emitted lazily (prefetch distance 1) so the
        # staging bursts interleave with rhs loads instead of queueing all
        # 45MB up front. One DMA per (ci,kd,k), 3 kh slots each.
        b27s = [None] * 6

        def stage(k, pieces=range(9)):
            Ek = 16 if k < 5 else 12
            if b27s[k] is None:
                bt = dramp.tile([27, Ek, HW27], F16, tag=f"B27_{k}")
                b27s[k] = bt
            bt = b27s[k]
            for p in pieces:
                ci, kd = p // 3, p % 3
                sap = bass.AP(
                    tensor=xh,
                    offset=ci * S3 + (kd + 16 * k) * S2,
                    ap=[[S, 3], [S2, Ek], [1, HW27]],
                )
                eng = nc.scalar if (p % 2) else nc.sync
                eng.dma_start(
                    out=bt[3 * p:3 * p + 3],
                    in_=sap)

        stage(0)

        for dq0 in (0, 4, 8, 12, 16, 20):
            ndq = min(4, Q - dq0)
            E = 4 * ndq
            bk = b27s[dq0 // 4]
            Ek = 16 if dq0 < 20 else 12
            for hq0 in (0, 8, 16):
                nq = min(8, Q - hq0)          # 8, 8, 7 h-quads in this chunk
                na1 = nq - 4                  # quads covered by col group a=1
                # rhs im2col tiles [108, E, 384], one 27-part DMA per quad
                rhs = [None, None]
                for a in (0, 1):
                    t = rhsp.tile([108, 16, 4 * S], F16, tag=f"rhs{hq0}{a}")
                    rhs[a] = t
                    gs = 4 if a == 0 else na1
                    for g in range(gs):
                        hq = hq0 + 4 * a + g
                        src = bass.AP(
                            tensor=bk.tensor,
                            offset=bk.offset + (4 * hq) * S,
                            ap=[[Ek * HW27, 27], [HW27, E], [1, 4 * S]],
                        )
                        eng = nc.scalar if (g % 2) else nc.sync
                        eng.dma_start(out=t[27 * g:27 * g + 27, 0:E, :],
                                      in_=src)
                if dq0 < 20:
                    # spread next d-chunk's staging: 3 pieces per hq0 block
                    stage(dq0 // 4 + 1, range(3 * (hq0 // 8), 3 * (hq0 // 8) + 3))

                for dq in range(dq0, dq0 + ndq):
                    hp = hpp.tile([128, 4, Q], F32)
                    for jh in (0, 1):
                      Ls = [None] * 2
                      s8 = pss.tile([128, 512], F32)
                      for jl in range(2):
                        j = 2 * jh + jl
                        dsi = 4 * (dq - dq0) + j
                        # --- conv: 3 kw taps x 2 col groups ---
                        L = psl.tile([128, 512], F32)
                        Ls[jl] = L
                        for kw in range(3):
                            for a in (0, 1):
                                ka = 108 if (a == 0 or na1 == 4) else 81
                                r = rhs[a][0:ka, dsi, :].rearrange(
                                    "p (hl w) -> p hl w", hl=4,
                                )[:, :, kw:kw + DU]
                                nc.tensor.matmul(
                                    out=L[64 * a:64 * a + 64, 0:368],
                                    lhsT=wlt[0:ka, kw, :],
                                    rhs=r,
                                    start=(kw == 0),
                                    stop=(kw == 2),
                                    skip_group_check=True,
                                )
                        if stage == 'conv':
                            continue
                        # --- exp(z + b) -> bf16 ---
                        e = ep.tile([128, 368], BF16)
                        nc.scalar.activation(
                            out=e, in_=L[:, 0:368],
                            func=mybir.ActivationFunctionType.Exp,
                            bias=biast[:, 0:1],
                        )
                        # --- group sums, 4x-duplicated rows 32jl..32jl+32 ---
                        nc.tensor.matmul(
                            out=s8[32 * jl:32 * jl + 32, 0:368],
                            lhsT=wst,
                            rhs=e,
                            start=True, stop=True,
                            skip_group_check=True,
                            tile_position=(0, 32 * jl),
                        )
                      if stage in ('conv', 'sum'):
                        continue
                      # --- ln(lambda*S): hi (f16) + optional exact (f32) ---
                      hl16 = hlp.tile([64, 368], F16)
                      nc.scalar.activation(
                          out=hl16, in_=s8[0:64, 0:368],
                          func=mybir.ActivationFunctionType.Ln,
                      )
                      if prec == 'hilo':
                        lnf = lnp.tile([64, 368], F32)
                        nc.scalar.activation(
                            out=lnf, in_=s8[0:64, 0:368],
                            func=mybir.ActivationFunctionType.Ln,
                        )
                        # lo = ln - hi written in place over rows 32jl..+8
                        # (rows +8..16 keep hi; DVE partition bases must be
                        # 32-aligned, and the bc coefficients are the same
                        # for both 8-row halves, so [lo; hi] order is fine)
                        for jl in range(2):
                            nc.vector.tensor_tensor(
                                out=hl16[32 * jl:32 * jl + 8, :],
                                in0=lnf[32 * jl:32 * jl + 8, :],
                                in1=hl16[32 * jl:32 * jl + 8, :],
                                op=mybir.AluOpType.subtract,
                            )
                      if stage == 'ln':
                        continue
                      for jl in range(2):
                        j = 2 * jh + jl
                        # --- y = z - ln(lambda*S): K<=16 accumulate matmul ---
                        kb = 16 if prec == 'hilo' else 8
                        nc.tensor.matmul(
                            out=Ls[jl][:, 0:368],
                            lhsT=bct[32 * jl:32 * jl + kb, :],
                            rhs=hl16[32 * jl:32 * jl + kb, :],
                            start=False, stop=True,
                            skip_group_check=True,
                            tile_position=(32 * jl, 0),
                        )
                        # --- maxpool over (hl, wl) -> hp[:, j, :] ---
                        nc.vector.reduce_max(
                            out=hp[:, j, :],
                            in_=Ls[jl][:, 0:368].rearrange(
                                "p (hl wq wl) -> p wq hl wl", hl=4, wq=Q),
                            axis=mybir.AxisListType.XY,
                        )
                    if stage in ('conv', 'sum', 'ln', 'pool1'):
                        continue
                    # --- maxpool over d, back to prob domain, store ---
                    fin = finp.tile([128, Q], F32)
                    nc.vector.reduce_max(
                        out=fin,
                        in_=hp.rearrange("p j w -> p w j"),
                        axis=mybir.AxisListType.X,
                    )
                    ot = outp.tile([128, Q], F32)
                    nc.scalar.activation(
                        out=ot, in_=fin,
                        func=mybir.ActivationFunctionType.Exp,
                        bias=biast2[:, 0:1],
                    )
                    nc.sync.dma_start(
                        out=out_[:][:, dq, hq0:hq0 + nq, :].rearrange(
                            "c g w -> g c w"),
                        in_=ot[0:16 * nq, :],
                    )


def _build(stage='full', prec='hilo'):
    nc = bacc.Bacc(name="conv_softmax_pool")
    xh = nc.declare_dram_parameter("xh", [CIN, S, S, S], F16, isOutput=False)
    wl_ = nc.declare_dram_parameter("wl", [108, 3, 64], F16, isOutput=False)
    ws_ = nc.declare_dram_parameter("ws", [128, 32], BF16, isOutput=False)
    bc_ = nc.declare_dram_parameter("bc", [128, 128], F16, isOutput=False)
    bias_ = nc.declare_dram_parameter("bias", [128, 1], F32, isOutput=False)
    bias2_ = nc.declare_dram_parameter("bias2", [128, 1], F32, isOutput=False)
    out_ = nc.declare_dram_parameter("out", [COUT, Q, Q, Q], F32, isOutput=True)
    _emit(nc, xh, wl_, ws_, bc_, bias_, bias2_, out_, stage=stage, prec=prec)
    nc.finalize()
    return nc


def _host_prep(w, b):
    """Build lhsT block-diagonal weights and softmax helper matrices."""
    # wl[r, kw, m]: r = 27g + 9ci + 3kd + kh, m = 16g + c  (g = 0..3)
    wh = w.astype(np.float32).astype(np.float16)
    wl = np.zeros((108, 3, 64), np.float16)
    for g in range(4):
        for ci in range(CIN):
            for kd in range(3):
                for kh in range(3):
                    wl[27 * g + 9 * ci + 3 * kd + kh, :, 16 * g:16 * g + 16] = \
                        wh[:, ci, kd, kh, :].T

    # ws32: lambda * group indicator, 4x duplicated along cols so PSUM rows
    # 32j..32j+32 all hold S (rows +8..16 feed the lo computation).
    # lambda = 2^-38 keeps ln's input inside the ACT Ln LUT's valid range.
    lam = np.float32(2.0 ** -38)
    ws = np.zeros((128, 32), np.float32)
    for p in range(128):
        for k in range(32):
            if p // 16 == k % 8:
                ws[p, k] = lam
    # bc: -(group indicator) for the K=16 (hi;lo) subtract matmul,
    # replicated at row offsets 0/32/64/96.
    bc = np.zeros((128, 128), np.float16)
    for j in range(4):
        for k in range(16):
            for p in range(128):
                if p // 16 == k % 8:
                    bc[32 * j + k, p] = -1.0
    bias = np.tile(b.astype(np.float32), 8).reshape(128, 1)
    # y = z - ln(lambda*S) = z - ln S + 38ln2; final exp needs
    # bias2 = b - 38ln2 to recover exp(z + b - ln S).
    bias2 = bias - np.float32(38.0 * np.log(2.0))
    return wl, ws.astype(np.float32), bc, bias, bias2


PREC = "hilo"


def kernel(x, w, b):
    key = ("nc", PREC)
    if key not in _cache:
        _cache[key] = _build(prec=PREC)
    nc = _cache[key]

    x = np.asarray(x, np.float32)
    w = np.asarray(w, np.float32)
    b = np.asarray(b, np.float32)
    wl, ws, bc, bias, bias2 = _host_prep(w, b)
    import ml_dtypes
    ws_bf16 = ws.astype(ml_dtypes.bfloat16)

    in_maps = []
    for i in range(N_CORES):
        m = {
            "xh": np.ascontiguousarray(x[i].astype(np.float16)),
            "wl": wl,
            "ws": ws_bf16,
            "bc": bc,
            "bias": bias,
            "bias2": bias2,
        }
        in_maps.append(m)

    res = run_bass_kernel_spmd(nc, in_maps, core_ids=list(range(N_CORES)))
    return np.stack([r["out"] for r in res.results]).astype(np.float32)


# revision 20
# speedup vs baseline: 2.2381x; 1.0377x over previous
"""Trainium2 Bass kernel: conv3d(16ch,3x3x3,VALID) -> channel softmax -> 2x maxpool3d(2).

Full inputs: x [8,3,96,96,96] f32, w [16,3,3,3,3] f32, b [16] f32.
Output: [8,16,23,23,23] f32.  Data-parallel: 1 sample per NeuronCore.

Per-core pipeline (sample x_i [3,96,96,96] -> out_i [16,23,23,23]):
  Only conv outputs d,h,w in [0,92) survive the two maxpools (23*4=92).
  Partition layout: p = 16*g + c for 8 h-quads g and 16 channels c.
  Free layout per conv depth d: (hl 4, w 92) -> N=368 per matmul.

  conv: single-term fp16 (x, w both rounded to fp16; measured end-to-end
  max err 4.2e-3 vs the 2e-2 gate). lhsT [108,64] holds 4 diagonal
  [27,16] blocks (K = 27 taps (ci,kd,kh) per quad); kw handled by 3
  PSUM-accumulated matmuls on column-shifted rhs views. Two col-group
  matmuls (tile cols 0-63 / 64-127) cover 8 quads. Chunks of h-quads:
  (0,8,16); the last chunk has 7 quads and relies on lhsT[0:81,:,0:64]
  having zero cols 48-63 so partitions 112-127 get zeros, not stale PSUM.

  softmax+pool in log domain: y = (z+b) - ln S, maxpool(softmax) =
  exp(maxpool(y)).
    exp:  ACT e = exp(z + b) -> SBUF bf16
    sum:  PE lhsT ws32 [128,32] bf16 (lambda=2^-38 group indicators,
          4x duplicated cols) -> S at PSUM rows 32j..32j+32 for d-slice j.
          One [128,512] PSUM tile collects all 4 slices of a d-quad.
    ln:   two ACT Ln over the packed [128,368] tile: ->f16 gives hi,
          ->f32 gives the exact value; DVE subtract writes lo = ln-hi
          into rows 32j+8..16 (valid because S was duplicated there).
    sub:  one K=16 f16 matmul per slice accumulates -(hi+lo) onto the
          logits PSUM (lhsT bc16: -1 group indicators, rows 32j..32j+16).
    pool: DVE reduce_max over (hl,wl) per slice -> hp[:,j,:], then
          reduce over j -> fin [128,23]
    out:  ACT exp(fin + b - 38*ln2), one 3-dim DMA per (dq, chunk).

  DMA: x is staged once (9 HBM->HBM copies) into B27[27,94,8928] f16 with
  B27[9ci+3kd+kh][d][j] = x[ci,kd+d, kh*96+j]; each rhs im2col tile
  quarter then loads as one 27-partition DMA (innermost 768B, full rate).
"""

import numpy as np
from contextlib import ExitStack

import concourse.bass as bass
import concourse.bacc as bacc
import concourse.tile as tile
from concourse import mybir
from concourse.bass_utils import run_bass_kernel_spmd

F32 = mybir.dt.float32
BF16 = mybir.dt.bfloat16
F16 = mybir.dt.float16

N_CORES = 8
CIN, S = 3, 96
COUT = 16
Q = 23          # pooled output size per dim
DU = 92         # conv positions used per dim (23*4)
S2 = S * S      # 9216
S3 = S * S * S
HW27 = 93 * 96  # 8928: flattened (h,w) span per staged slot
BD = 94         # staged depth extent

_cache: dict = {}


def _emit(nc, xh, wl_, ws_, bc_, bias_, bias2_, out_, stage='full', prec='hilo'):
    # Preload the one ACT table set containing BOTH Exp and Ln; the
    # insert_act_table_loads pass then sees every activation covered and
    # emits no per-switch reloads (saves ~1.3us x ~276 switches).
    from concourse.hw_specs import get_activation_tables
    _tabs = list(get_activation_tables(nc.m.arch).keys())
    nc.scalar.add_instruction(mybir.InstLoadActFuncSet(
        name="preload_act",
        act_func_set_id=_tabs.index("natural_log_exp_and_others"),
        ins=[], outs=[]))
    with tile.TileContext(nc) as tc, ExitStack() as ctx:
        consts = ctx.enter_context(tc.tile_pool(name="consts", bufs=1))
        rhsp = ctx.enter_context(tc.tile_pool(name="rhs", bufs=2))
        ep = ctx.enter_context(tc.tile_pool(name="e", bufs=6))
        hlp = ctx.enter_context(tc.tile_pool(name="hl", bufs=2))
        lnp = ctx.enter_context(tc.tile_pool(name="ln", bufs=2))
        hpp = ctx.enter_context(tc.tile_pool(name="hp", bufs=2))
        finp = ctx.enter_context(tc.tile_pool(name="fin", bufs=2))
        outp = ctx.enter_context(tc.tile_pool(name="outt", bufs=2))
        psl = ctx.enter_context(tc.tile_pool(name="psl", bufs=6, space="PSUM"))
        pss = ctx.enter_context(tc.tile_pool(name="pss", bufs=2, space="PSUM"))
        dramp = ctx.enter_context(tc.tile_pool(name="dram", bufs=1, space="DRAM"))

        # constants
        wlt = consts.tile([108, 3, 64], F16, tag="wl")
        nc.sync.dma_start(out=wlt, in_=wl_[:])
        wst = consts.tile([128, 32], BF16, tag="ws")
        nc.sync.dma_start(out=wst, in_=ws_[:])
        bct = consts.tile([128, 128], F16, tag="bc")
        nc.sync.dma_start(out=bct, in_=bc_[:])
        biast = consts.tile([128, 1], F32, tag="bias")
        nc.sync.dma_start(out=biast, in_=bias_[:])
        biast2 = consts.tile([128, 1], F32, tag="bias2")
        nc.sync.dma_start(out=biast2, in_=bias2_[:])

        # --- staging: B27_k[9ci+3kd+kh, d, j] = xh[ci, kd+16k+d, kh*96 + j]
        # split into 6 d-chunk tiles so rhs loads of group k only wait for
        # chunk k's staging; emitted lazily (prefetch distance 1) so the
        # staging bursts interleave with rhs loads instead of queueing all
        # 45MB up front. One DMA per (ci,kd,k), 3 kh slots each.
        b27s = [None] * 6

        def stage(k, pieces=range(9)):
            Ek = 16 if k < 5 else 12
            if b27s[k] is None:
                bt = dramp.tile([27, Ek, HW27], F16, tag=f"B27_{k}")
                b27s[k] = bt
            bt = b27s[k]
            for p in pieces:
                ci, kd = p // 3, p % 3
                sap = bass.AP(
                    tensor=xh,
                    offset=ci * S3 + (kd + 16 * k) * S2,
                    ap=[[S, 3], [S2, Ek], [1, HW27]],
                )
                eng = nc.scalar if (p % 2) else nc.sync
                eng.dma_start(
                    out=bt[3 * p:3 * p + 3],
                    in_=sap)

        stage(0)

        for dq0 in (0, 4, 8, 12, 16, 20):
            ndq = min(4, Q - dq0)
            E = 4 * ndq
            bk = b27s[dq0 // 4]
            Ek = 16 if dq0 < 20 else 12
            for hq0 in (0, 8, 16):
                nq = min(8, Q - hq0)          # 8, 8, 7 h-quads in this chunk
                na1 = nq - 4                  # quads covered by col group a=1
                # rhs im2col tiles [108, E, 384], one 27-part DMA per quad
                rhs = [None, None]
                for a in (0, 1):
                    t = rhsp.tile([108, 16, 4 * S], F16, tag=f"rhs{hq0}{a}")
                    rhs[a] = t
                    gs = 4 if a == 0 else na1
                    for g in range(gs):
                        hq = hq0 + 4 * a + g
                        src = bass.AP(
                            tensor=bk.tensor,
                            offset=bk.offset + (4 * hq) * S,
                            ap=[[Ek * HW27, 27], [HW27, E], [1, 4 * S]],
                        )
                        eng = nc.scalar if (g % 2) else nc.sync
                        eng.dma_start(out=t[27 * g:27 * g + 27, 0:E, :],
                                      in_=src)
                if dq0 < 20:
                    # spread next d-chunk's staging: 3 pieces per hq0 block
                    stage(dq0 // 4 + 1, range(3 * (hq0 // 8), 3 * (hq0 // 8) + 3))

                for dq in range(dq0, dq0 + ndq):
                    hp = hpp.tile([128, 4, Q], F32)
                    for jh in (0, 1):
                      Ls = [None] * 2
                      s8 = pss.tile([128, 512], F32)
                      for jl in range(2):
                        j = 2 * jh + jl
                        dsi = 4 * (dq - dq0) + j
                        # --- conv: 3 kw taps x 2 col groups ---
                        L = psl.tile([128, 512], F32)
                        Ls[jl] = L
                        for kw in range(3):
                            for a in (0, 1):
                                ka = 108 if (a == 0 or na1 == 4) else 81
                                r = rhs[a][0:ka, dsi, :].rearrange(
                                    "p (hl w) -> p hl w", hl=4,
                                )[:, :, kw:kw + DU]
                                nc.tensor.matmul(
                                    out=L[64 * a:64 * a + 64, 0:368],
                                    lhsT=wlt[0:ka, kw, :],
                                    rhs=r,
                                    start=(kw == 0),
                                    stop=(kw == 2),
                                    skip_group_check=True,
                                )
                        if stage == 'conv':
                            continue
                        # --- exp(z + b) -> bf16 ---
                        e = ep.tile([128, 368], BF16)
                        nc.scalar.activation(
                            out=e, in_=L[:, 0:368],
                            func=mybir.ActivationFunctionType.Exp,
                            bias=biast[:, 0:1],
                        )
                        # --- group sums, 4x-duplicated rows 32jl..32jl+32 ---
                        nc.tensor.matmul(
                            out=s8[32 * jl:32 * jl + 32, 0:368],
                            lhsT=wst,
                            rhs=e,
                            start=True, stop=True,
                            skip_group_check=True,
                            tile_position=(0, 32 * jl),
                        )
                      if stage in ('conv', 'sum'):
                        continue
                      # --- ln(lambda*S): hi (f16) + optional exact (f32) ---
                      hl16 = hlp.tile([64, 368], F16)
                      nc.scalar.activation(
                          out=hl16, in_=s8[0:64, 0:368],
                          func=mybir.ActivationFunctionType.Ln,
                      )
                      if prec == 'hilo':
                        lnf = lnp.tile([64, 368], F32)
                        nc.scalar.activation(
                            out=lnf, in_=s8[0:64, 0:368],
                            func=mybir.ActivationFunctionType.Ln,
                        )
                        # lo = ln - hi written in place over rows 32jl..+8
                        # (rows +8..16 keep hi; DVE partition bases must be
                        # 32-aligned, and the bc coefficients are the same
                        # for both 8-row halves, so [lo; hi] order is fine)
                        for jl in range(2):
                            nc.vector.tensor_tensor(
                                out=hl16[32 * jl:32 * jl + 8, :],
                                in0=lnf[32 * jl:32 * jl + 8, :],
                                in1=hl16[32 * jl:32 * jl + 8, :],
                                op=mybir.AluOpType.subtract,
                            )
                      if stage == 'ln':
                        continue
                      for jl in range(2):
                        j = 2 * jh + jl
                        # --- y = z - ln(lambda*S): K<=16 accumulate matmul ---
                        kb = 16 if prec == 'hilo' else 8
                        nc.tensor.matmul(
                            out=Ls[jl][:, 0:368],
                            lhsT=bct[32 * jl:32 * jl + kb, :],
                            rhs=hl16[32 * jl:32 * jl + kb, :],
                            start=False, stop=True,
                            skip_group_check=True,
                            tile_position=(32 * jl, 0),
                        )
                        # --- maxpool over (hl, wl) -> hp[:, j, :] ---
                        nc.vector.reduce_max(
                            out=hp[:, j, :],
                            in_=Ls[jl][:, 0:368].rearrange(
                                "p (hl wq wl) -> p wq hl wl", hl=4, wq=Q),
                            axis=mybir.AxisListType.XY,
                        )
                    if stage in ('conv', 'sum', 'ln', 'pool1'):
                        continue
                    # --- maxpool over d, back to prob domain, store ---
                    fin = finp.tile([128, Q], F32)
                    nc.vector.reduce_max(
                        out=fin,
                        in_=hp.rearrange("p j w -> p w j"),
                        axis=mybir.AxisListType.X,
                    )
                    ot = outp.tile([128, Q], F32)
                    nc.scalar.activation(
                        out=ot, in_=fin,
                        func=mybir.ActivationFunctionType.Exp,
                        bias=biast2[:, 0:1],
                    )
                    nc.sync.dma_start(
                        out=out_[:][:, dq, hq0:hq0 + nq, :].rearrange(
                            "c g w -> g c w"),
                        in_=ot[0:16 * nq, :],
                    )


def _build(stage='full', prec='hilo'):
    nc = bacc.Bacc(name="conv_softmax_pool")
    xh = nc.declare_dram_parameter("xh", [CIN, S, S, S], F16, isOutput=False)
    wl_ = nc.declare_dram_parameter("wl", [108, 3, 64], F16, isOutput=False)
    ws_ = nc.declare_dram_parameter("ws", [128, 32], BF16, isOutput=False)
    bc_ = nc.declare_dram_parameter("bc", [128, 128], F16, isOutput=False)
    bias_ = nc.declare_dram_parameter("bias", [128, 1], F32, isOutput=False)
    bias2_ = nc.declare_dram_parameter("bias2", [128, 1], F32, isOutput=False)
    out_ = nc.declare_dram_parameter("out", [COUT, Q, Q, Q], F32, isOutput=True)
    _emit(nc, xh, wl_, ws_, bc_, bias_, bias2_, out_, stage=stage, prec=prec)
    nc.finalize()
    return nc


def _host_prep(w, b):
    """Build lhsT block-diagonal weights and softmax helper matrices."""
    # wl[r, kw, m]: r = 27g + 9ci + 3kd + kh, m = 16g + c  (g = 0..3)
    wh = w.astype(np.float32).astype(np.float16)
    wl = np.zeros((108, 3, 64), np.float16)
    for g in range(4):
        for ci in range(CIN):
            for kd in range(3):
                for kh in range(3):
                    wl[27 * g + 9 * ci + 3 * kd + kh, :, 16 * g:16 * g + 16] = \
                        wh[:, ci, kd, kh, :].T

    # ws32: lambda * group indicator, 4x duplicated along cols so PSUM rows
    # 32j..32j+32 all hold S (rows +8..16 feed the lo computation).
    # lambda = 2^-38 keeps ln's input inside the ACT Ln LUT's valid range.
    lam = np.float32(2.0 ** -38)
    ws = np.zeros((128, 32), np.float32)
    for p in range(128):
        for k in range(32):
            if p // 16 == k % 8:
                ws[p, k] = lam
    # bc: -(group indicator) for the K=16 (hi;lo) subtract matmul,
    # replicated at row offsets 0/32/64/96.
    bc = np.zeros((128, 128), np.float16)
    for j in range(4):
        for k in range(16):
            for p in range(128):
                if p // 16 == k % 8:
                    bc[32 * j + k, p] = -1.0
    bias = np.tile(b.astype(np.float32), 8).reshape(128, 1)
    # y = z - ln(lambda*S) = z - ln S + 38ln2; final exp needs
    # bias2 = b - 38ln2 to recover exp(z + b - ln S).
    bias2 = bias - np.float32(38.0 * np.log(2.0))
    return wl, ws.astype(np.float32), bc, bias, bias2


PREC = "f16"


def kernel(x, w, b):
    key = ("nc", PREC)
    if key not in _cache:
        _cache[key] = _build(prec=PREC)
    nc = _cache[key]

    x = np.asarray(x, np.float32)
    w = np.asarray(w, np.float32)
    b = np.asarray(b, np.float32)
    wl, ws, bc, bias, bias2 = _host_prep(w, b)
    import ml_dtypes
    ws_bf16 = ws.astype(ml_dtypes.bfloat16)

    in_maps = []
    for i in range(N_CORES):
        m = {
            "xh": np.ascontiguousarray(x[i].astype(np.float16)),
            "wl": wl,
            "ws": ws_bf16,
            "bc": bc,
            "bias": bias,
            "bias2": bias2,
        }
        in_maps.append(m)

    res = run_bass_kernel_spmd(nc, in_maps, core_ids=list(range(N_CORES)))
    return np.stack([r["out"] for r in res.results]).astype(np.float32)


# revision 25
# speedup vs baseline: 2.6991x; 1.2060x over previous
"""Trainium2 Bass kernel: conv3d(16ch,3x3x3,VALID) -> channel softmax -> 2x maxpool3d(2).

Full inputs: x [8,3,96,96,96] f32, w [16,3,3,3,3] f32, b [16] f32.
Output: [8,16,23,23,23] f32.  Data-parallel: 1 sample per NeuronCore.

Per-core pipeline (sample x_i [3,96,96,96] -> out_i [16,23,23,23]):
  Only conv outputs d,h,w in [0,92) survive the two maxpools (23*4=92).
  Partition layout: p = 16*g + c for 8 h-quads g and 16 channels c.
  Free layout per conv depth d: (hl 4, w 92) -> N=368 per matmul.

  conv: single-term fp16 (x, w both rounded to fp16; measured end-to-end
  max err 4.2e-3 vs the 2e-2 gate). lhsT [108,64] holds 4 diagonal
  [27,16] blocks (K = 27 taps (ci,kd,kh) per quad); kw handled by 3
  PSUM-accumulated matmuls on column-shifted rhs views. Two col-group
  matmuls (tile cols 0-63 / 64-127) cover 8 quads. Chunks of h-quads:
  (0,8,16); the last chunk has 7 quads and relies on lhsT[0:81,:,0:64]
  having zero cols 48-63 so partitions 112-127 get zeros, not stale PSUM.

  softmax+pool in log domain: y = (z+b) - ln S, maxpool(softmax) =
  exp(maxpool(y)).
    exp:  ACT e = exp(z + b) -> SBUF bf16
    sum:  PE lhsT ws32 [128,32] bf16 (lambda=2^-38 group indicators,
          4x duplicated cols) -> S at PSUM rows 32j..32j+32 for d-slice j.
          One [128,512] PSUM tile collects all 4 slices of a d-quad.
    ln:   two ACT Ln over the packed [128,368] tile: ->f16 gives hi,
          ->f32 gives the exact value; DVE subtract writes lo = ln-hi
          into rows 32j+8..16 (valid because S was duplicated there).
    sub:  one K=16 f16 matmul per slice accumulates -(hi+lo) onto the
          logits PSUM (lhsT bc16: -1 group indicators, rows 32j..32j+16).
    pool: DVE reduce_max over (hl,wl) per slice -> hp[:,j,:], then
          reduce over j -> fin [128,23]
    out:  ACT exp(fin + b - 38*ln2), one 3-dim DMA per (dq, chunk).

  DMA: x is staged once (9 HBM->HBM copies) into B27[27,94,8928] f16 with
  B27[9ci+3kd+kh][d][j] = x[ci,kd+d, kh*96+j]; each rhs im2col tile
  quarter then loads as one 27-partition DMA (innermost 768B, full rate).
"""

import numpy as np
from contextlib import ExitStack

import concourse.bass as bass
import concourse.bacc as bacc
import concourse.tile as tile
from concourse import mybir
from concourse.bass_utils import run_bass_kernel_spmd

F32 = mybir.dt.float32
BF16 = mybir.dt.bfloat16
F16 = mybir.dt.float16

N_CORES = 8
CIN, S = 3, 96
COUT = 16
Q = 23          # pooled output size per dim
DU = 92         # conv positions used per dim (23*4)
S2 = S * S      # 9216
S3 = S * S * S
HW27 = 93 * 96  # 8928: flattened (h,w) span per staged slot
BD = 94         # staged depth extent

_cache: dict = {}


def _emit(nc, xh, wl_, ws_, bc_, bias_, bias2_, out_, stage='full', prec='hilo'):
    # Preload the one ACT table set containing BOTH Exp and Ln; the
    # insert_act_table_loads pass then sees every activation covered and
    # emits no per-switch reloads (saves ~1.3us x ~276 switches).
    from concourse.hw_specs import get_activation_tables
    _tabs = list(get_activation_tables(nc.m.arch).keys())
    nc.scalar.add_instruction(mybir.InstLoadActFuncSet(
        name="preload_act",
        act_func_set_id=_tabs.index("natural_log_exp_and_others"),
        ins=[], outs=[]))
    with tile.TileContext(nc) as tc, ExitStack() as ctx:
        consts = ctx.enter_context(tc.tile_pool(name="consts", bufs=1))
        rhsp = ctx.enter_context(tc.tile_pool(name="rhs", bufs=3))
        ep = ctx.enter_context(tc.tile_pool(name="e", bufs=6))
        hlp = ctx.enter_context(tc.tile_pool(name="hl", bufs=2))
        lnp = ctx.enter_context(tc.tile_pool(name="ln", bufs=2))
        hpp = ctx.enter_context(tc.tile_pool(name="hp", bufs=2))
        finp = ctx.enter_context(tc.tile_pool(name="fin", bufs=2))
        outp = ctx.enter_context(tc.tile_pool(name="outt", bufs=2))
        psl = ctx.enter_context(tc.tile_pool(name="psl", bufs=6, space="PSUM"))
        pss = ctx.enter_context(tc.tile_pool(name="pss", bufs=2, space="PSUM"))
        dramp = ctx.enter_context(tc.tile_pool(name="dram", bufs=1, space="DRAM"))

        # constants
        wlt = consts.tile([108, 3, 64], F16, tag="wl")
        nc.sync.dma_start(out=wlt, in_=wl_[:])
        wst = consts.tile([128, 32], BF16, tag="ws")
        nc.sync.dma_start(out=wst, in_=ws_[:])
        bct = consts.tile([128, 128], F16, tag="bc")
        nc.sync.dma_start(out=bct, in_=bc_[:])
        biast = consts.tile([128, 1], F32, tag="bias")
        nc.sync.dma_start(out=biast, in_=bias_[:])
        biast2 = consts.tile([128, 1], F32, tag="bias2")
        nc.sync.dma_start(out=biast2, in_=bias2_[:])

        # --- staging: B9_k[3ci+kh, d', j] = xh[ci, 32k+d', kh*96 + j] ---
        # Only the kh replication is staged (3x bytes, not 8.5x); kd is
        # handled by a free-dim d-offset in the rhs loads. 3 d-chunks with a
        # +2 halo; staging pieces are interleaved with rhs loads so the DMA
        # FIFO never sees a long staging burst.
        DKS = (34, 34, 30)
        b9s = [None] * 3

        def stage(k, pieces=range(9)):
            Dk = DKS[k]
            if b9s[k] is None:
                bt = dramp.tile([9, Dk, HW27], F16, tag=f"B9_{k}")
                b9s[k] = bt
            bt = b9s[k]
            for p in pieces:
                ci, kh = p // 3, p % 3
                sap = bass.AP(
                    tensor=xh,
                    offset=ci * S3 + 32 * k * S2 + kh * S,
                    ap=[[S2, Dk], [1, HW27]],
                )
                eng = (nc.sync, nc.scalar, nc.gpsimd)[p % 3]
                eng.dma_start(out=bt[p], in_=sap)

        stage(0)

        for dq0 in (0, 8, 16):
            ndq = min(8, Q - dq0)
            E = 4 * ndq
            kc = dq0 // 8
            bk = b9s[kc]
            Dk = DKS[kc]
            for hq0 in (0, 8, 16):
                nq = min(8, Q - hq0)          # 8, 8, 7 h-quads in this chunk
                na1 = nq - 4                  # quads covered by col group a=1
                # rhs im2col tiles [108, E, 384]: row 27g+9kd+3ci+kh reads
                # B9[3ci+kh] at d-offset +kd -> one 9-part DMA per (g, kd)
                rhs = [None, None]
                for a in (0, 1):
                    t = rhsp.tile([108, 32, 4 * S], F16, tag=f"rhs{a}")
                    rhs[a] = t
                    gs = 4 if a == 0 else na1
                    for g in range(gs):
                        hq = hq0 + 4 * a + g
                        for kd in range(3):
                            src = bass.AP(
                                tensor=bk.tensor,
                                offset=(bk.offset + kd * HW27
                                        + (4 * hq) * S),
                                ap=[[Dk * HW27, 9], [HW27, E], [1, 4 * S]],
                            )
                            eng = (nc.sync, nc.scalar, nc.gpsimd)[
                                (3 * g + kd) % 3]
                            eng.dma_start(
                                out=t[27 * g + 9 * kd:27 * g + 9 * kd + 9,
                                      0:E, :],
                                in_=src)
                if dq0 < 16:
                    # spread next d-chunk's staging: 3 pieces per hq0 block
                    stage(dq0 // 8 + 1, range(3 * (hq0 // 8), 3 * (hq0 // 8) + 3))

                for dq in range(dq0, dq0 + ndq):
                    hp = hpp.tile([128, 4, Q], F32)
                    for jh in (0, 1):
                      Ls = [None] * 2
                      s8 = pss.tile([128, 512], F32)
                      for jl in range(2):
                        j = 2 * jh + jl
                        dsi = 4 * (dq - dq0) + j
                        # --- conv: 3 kw taps x 2 col groups ---
                        L = psl.tile([128, 512], F32)
                        Ls[jl] = L
                        for kw in range(3):
                            for a in (0, 1):
                                ka = 108 if (a == 0 or na1 == 4) else 81
                                r = rhs[a][0:ka, dsi, :].rearrange(
                                    "p (hl w) -> p hl w", hl=4,
                                )[:, :, kw:kw + DU]
                                nc.tensor.matmul(
                                    out=L[64 * a:64 * a + 64, 0:368],
                                    lhsT=wlt[0:ka, kw, :],
                                    rhs=r,
                                    start=(kw == 0),
                                    stop=(kw == 2),
                                    skip_group_check=True,
                                )
                        if stage == 'conv':
                            continue
                        # --- exp(z + b) -> bf16 ---
                        e = ep.tile([128, 368], BF16)
                        nc.scalar.activation(
                            out=e, in_=L[:, 0:368],
                            func=mybir.ActivationFunctionType.Exp,
                            bias=biast[:, 0:1],
                        )
                        # --- group sums, 4x-duplicated rows 32jl..32jl+32 ---
                        nc.tensor.matmul(
                            out=s8[32 * jl:32 * jl + 32, 0:368],
                            lhsT=wst,
                            rhs=e,
                            start=True, stop=True,
                            skip_group_check=True,
                            tile_position=(0, 32 * jl),
                        )
                      if stage in ('conv', 'sum'):
                        continue
                      # --- ln(lambda*S): hi (f16) + optional exact (f32) ---
                      hl16 = hlp.tile([64, 368], F16)
                      nc.scalar.activation(
                          out=hl16, in_=s8[0:64, 0:368],
                          func=mybir.ActivationFunctionType.Ln,
                      )
                      if prec == 'hilo':
                        lnf = lnp.tile([64, 368], F32)
                        nc.scalar.activation(
                            out=lnf, in_=s8[0:64, 0:368],
                            func=mybir.ActivationFunctionType.Ln,
                        )
                        # lo = ln - hi written in place over rows 32jl..+8
                        # (rows +8..16 keep hi; DVE partition bases must be
                        # 32-aligned, and the bc coefficients are the same
                        # for both 8-row halves, so [lo; hi] order is fine)
                        for jl in range(2):
                            nc.vector.tensor_tensor(
                                out=hl16[32 * jl:32 * jl + 8, :],
                                in0=lnf[32 * jl:32 * jl + 8, :],
                                in1=hl16[32 * jl:32 * jl + 8, :],
                                op=mybir.AluOpType.subtract,
                            )
                      if stage == 'ln':
                        continue
                      for jl in range(2):
                        j = 2 * jh + jl
                        # --- y = z - ln(lambda*S): K<=16 accumulate matmul ---
                        kb = 16 if prec == 'hilo' else 8
                        nc.tensor.matmul(
                            out=Ls[jl][:, 0:368],
                            lhsT=bct[32 * jl:32 * jl + kb, :],
                            rhs=hl16[32 * jl:32 * jl + kb, :],
                            start=False, stop=True,
                            skip_group_check=True,
                            tile_position=(32 * jl, 0),
                        )
                        # --- maxpool over (hl, wl) -> hp[:, j, :] ---
                        nc.vector.reduce_max(
                            out=hp[:, j, :],
                            in_=Ls[jl][:, 0:368].rearrange(
                                "p (hl wq wl) -> p wq hl wl", hl=4, wq=Q),
                            axis=mybir.AxisListType.XY,
                        )
                    if stage in ('conv', 'sum', 'ln', 'pool1'):
                        continue
                    # --- maxpool over d, back to prob domain, store ---
                    fin = finp.tile([128, Q], F32)
                    nc.vector.reduce_max(
                        out=fin,
                        in_=hp.rearrange("p j w -> p w j"),
                        axis=mybir.AxisListType.X,
                    )
                    ot = outp.tile([128, Q], F32)
                    nc.scalar.activation(
                        out=ot, in_=fin,
                        func=mybir.ActivationFunctionType.Exp,
                        bias=biast2[:, 0:1],
                    )
                    # out-DMA via SWDGE (gpsimd): a sync/scalar-queued store
                    # would hold that SEQ while waiting on this dq's compute,
                    # blocking the next block's rhs loads queued behind it.
                    nc.gpsimd.dma_start(
                        out=out_[:][:, dq, hq0:hq0 + nq, :].rearrange(
                            "c g w -> g c w"),
                        in_=ot[0:16 * nq, :],
                    )


def _build(stage='full', prec='hilo'):
    nc = bacc.Bacc(name="conv_softmax_pool")
    xh = nc.declare_dram_parameter("xh", [CIN, S, S, S], F16, isOutput=False)
    wl_ = nc.declare_dram_parameter("wl", [108, 3, 64], F16, isOutput=False)
    ws_ = nc.declare_dram_parameter("ws", [128, 32], BF16, isOutput=False)
    bc_ = nc.declare_dram_parameter("bc", [128, 128], F16, isOutput=False)
    bias_ = nc.declare_dram_parameter("bias", [128, 1], F32, isOutput=False)
    bias2_ = nc.declare_dram_parameter("bias2", [128, 1], F32, isOutput=False)
    out_ = nc.declare_dram_parameter("out", [COUT, Q, Q, Q], F32, isOutput=True)
    _emit(nc, xh, wl_, ws_, bc_, bias_, bias2_, out_, stage=stage, prec=prec)
    nc.finalize()
    return nc


def _host_prep(w, b):
    """Build lhsT block-diagonal weights and softmax helper matrices."""
    # wl[r, kw, m]: r = 27g + 9ci + 3kd + kh, m = 16g + c  (g = 0..3)
    wh = w.astype(np.float32).astype(np.float16)
    wl = np.zeros((108, 3, 64), np.float16)
    for g in range(4):
        for ci in range(CIN):
            for kd in range(3):
                for kh in range(3):
                    wl[27 * g + 9 * kd + 3 * ci + kh, :, 16 * g:16 * g + 16] = \
                        wh[:, ci, kd, kh, :].T

    # ws32: lambda * group indicator, 4x duplicated along cols so PSUM rows
    # 32j..32j+32 all hold S (rows +8..16 feed the lo computation).
    # lambda = 2^-38 keeps ln's input inside the ACT Ln LUT's valid range.
    lam = np.float32(2.0 ** -38)
    ws = np.zeros((128, 32), np.float32)
    for p in range(128):
        for k in range(32):
            if p // 16 == k % 8:
                ws[p, k] = lam
    # bc: -(group indicator) for the K=16 (hi;lo) subtract matmul,
    # replicated at row offsets 0/32/64/96.
    bc = np.zeros((128, 128), np.float16)
    for j in range(4):
        for k in range(16):
            for p in range(128):
                if p // 16 == k % 8:
                    bc[32 * j + k, p] = -1.0
    bias = np.tile(b.astype(np.float32), 8).reshape(128, 1)
    # y = z - ln(lambda*S) = z - ln S + 38ln2; final exp needs
    # bias2 = b - 38ln2 to recover exp(z + b - ln S).
    bias2 = bias - np.float32(38.0 * np.log(2.0))
    return wl, ws.astype(np.float32), bc, bias, bias2


PREC = "f16"


def kernel(x, w, b):
    key = ("nc", PREC)
    if key not in _cache:
        _cache[key] = _build(prec=PREC)
    nc = _cache[key]

    x = np.asarray(x, np.float32)
    w = np.asarray(w, np.float32)
    b = np.asarray(b, np.float32)
    wl, ws, bc, bias, bias2 = _host_prep(w, b)
    import ml_dtypes
    ws_bf16 = ws.astype(ml_dtypes.bfloat16)

    in_maps = []
    for i in range(N_CORES):
        m = {
            "xh": np.ascontiguousarray(x[i].astype(np.float16)),
            "wl": wl,
            "ws": ws_bf16,
            "bc": bc,
            "bias": bias,
            "bias2": bias2,
        }
        in_maps.append(m)

    res = run_bass_kernel_spmd(nc, in_maps, core_ids=list(range(N_CORES)))
    return np.stack([r["out"] for r in res.results]).astype(np.float32)
